# revision 3
# baseline (speedup 1.0000x reference)
"""Trainium2 Bass kernel for a Neural CDE (GunnarODE).

Full-input contract: kernel(**inputs) takes the complete (unsharded) inputs
and returns the complete (L, B, Y) output. Internally the batch dim (B=256)
is sharded across 8 NeuronCores (32 per core); the small MLP weights are
replicated. The sequential 127-step x 2-substep Euler scan runs on-device.

v3 design notes:
- Zero per-substep DMA: spline-derivative scalars for all 254 substeps are
  resident in SBUF; the block-diag dX operand is rebuilt each substep on the
  (otherwise idle) GPSIMD engine.
- Software-pipelined PE queue: each substep's bias matmuls are emitted at
  the TOP of the iteration so they execute during the previous substep's
  tanh/transpose tail; contract matmuls are emitted after ALL mains so the
  PE FIFO never stalls behind the scalar engine mid-stream.
- Group-granular tanh chunks issue as soon as their mains retire; the
  singleton c-group (c16) is computed first so its tanh leaves the critical
  path. Contractions are split by h-half so the PSUM->SBUF copy and PE
  transposes pipeline.
- Channels unpadded (c=17 as [1,4,4,4,4] groups); gS/dX/decoder in bf16;
  fp32 master state with the rounded bf16 copy produced in one DVE op.
"""

import sys

for _p in ("/opt/trn_rl_repo", "/root/.axon_site/_ro/trn_rl_repo"):
    if _p not in sys.path:
        sys.path.append(_p)

import numpy as np
import ml_dtypes

import concourse.bass as bass
import concourse.bacc as bacc
import concourse.mybir as mybir
from concourse.tile import TileContext
from concourse.bass_utils import run_bass_kernel_spmd

# Problem dims (hardcoded per contract)
L, B, H, C, Y = 128, 256, 256, 17, 16
N_SUB = 2
NCORES = 8
BC = B // NCORES           # batch per core = 32
NSTEPS = L - 1             # 127 scan steps
NSUBSTEPS = NSTEPS * N_SUB # 254
NG = 5                     # c-groups: [4,4,4,4,1] (g4 = c16 singleton)
GF = 256                   # free width of one c-group block in G
F32 = mybir.dt.float32
BF16 = mybir.dt.bfloat16

AluOp = mybir.AluOpType
Act = mybir.ActivationFunctionType

# emission order of groups within a substep: singleton first so its tanh
# leaves the critical path, then the 4-c groups
GORDER = [4, 0, 1, 2, 3]
# tanh chunks as (lo, hi, [groups]) in gP free coordinates, issued in order
TANH_CHUNKS = [
    (4 * GF, 5 * GF, [4]),
    (0 * GF, 2 * GF, [0, 1]),
    (2 * GF, 4 * GF, [2, 3]),
]


def build_bass(n_substeps=NSUBSTEPS, warmup=40, dump=False):
    """Build the per-core Bass program (same program for all cores)."""
    nc = bacc.Bacc("TRN2", target_bir_lowering=False, debug=False)

    w1s = nc.dram_tensor("w1s", [128, 512], BF16, kind="ExternalInput")
    w2s = nc.dram_tensor("w2s", [128, 2 * C * 256], BF16, kind="ExternalInput")
    b2s = nc.dram_tensor("b2s", [128, C * 256], BF16, kind="ExternalInput")
    b1t = nc.dram_tensor("b1t", [128, 2], F32, kind="ExternalInput")
    zt0 = nc.dram_tensor("zt0", [128, 2 * BC], F32, kind="ExternalInput")
    dcol = nc.dram_tensor("dcol", [128, NSUBSTEPS * NG], F32, kind="ExternalInput")
    maskd = nc.dram_tensor("maskd", [128, NG * BC], BF16, kind="ExternalInput")
    ones = nc.dram_tensor("ones", [128, BC], BF16, kind="ExternalInput")
    ident = nc.dram_tensor("ident", [BC, BC], F32, kind="ExternalInput")
    decw = nc.dram_tensor("decw", [128, 2 * Y], BF16, kind="ExternalInput")
    decb = nc.dram_tensor("decb", [128, 1], F32, kind="ExternalInput")
    out = nc.dram_tensor("out", [L, BC, Y], F32, kind="ExternalOutput")
    if dump:
        dbg_hdn = nc.dram_tensor("dbg_hdn", [128, 2 * BC], F32, kind="ExternalOutput")
        dbg_g = nc.dram_tensor("dbg_g", [128, NG * GF], F32, kind="ExternalOutput")
        dbg_zd = nc.dram_tensor("dbg_zd", [BC, 256], F32, kind="ExternalOutput")
        dbg_zt = nc.dram_tensor("dbg_zt", [128, 2 * BC], F32, kind="ExternalOutput")

    with TileContext(nc) as tc:
        with (
            tc.tile_pool(name="const", bufs=1) as constp,
            tc.tile_pool(name="work", bufs=2) as work,
            tc.tile_pool(name="state", bufs=1) as statep,
            tc.tile_pool(name="psG", bufs=1, space="PSUM") as psG,
            tc.tile_pool(name="psH", bufs=1, space="PSUM") as psH,
            tc.tile_pool(name="psZ", bufs=1, space="PSUM") as psZ,
        ):
            w1 = constp.tile([128, 512], BF16)
            nc.sync.dma_start(w1[:], w1s[:])
            w2 = constp.tile([128, 2 * C * 256], BF16)
            nc.sync.dma_start(w2[:], w2s[:])
            b2 = constp.tile([128, C * 256], BF16)
            nc.sync.dma_start(b2[:], b2s[:])
            b1 = constp.tile([128, 2], F32)
            nc.sync.dma_start(b1[:], b1t[:])
            onest = constp.tile([128, BC], BF16)
            nc.sync.dma_start(onest[:], ones[:])
            identt = constp.tile([BC, BC], F32)
            nc.sync.dma_start(identt[:], ident[:])
            maskt = constp.tile([128, NG * BC], BF16)
            nc.sync.dma_start(maskt[:], maskd[:])
            dcolt = constp.tile([128, NSUBSTEPS * NG], F32)
            nc.sync.dma_start(dcolt[:], dcol[:])
            decwt = constp.tile([128, 2 * Y], BF16)
            nc.sync.dma_start(decwt[:], decw[:])
            decbt = constp.tile([128, 1], F32)
            nc.sync.dma_start(decbt[:], decb[:])

            zT = statep.tile([128, 2 * BC], F32)   # fp32 master state, transposed
            nc.sync.dma_start(zT[:], zt0[:])
            zTr = statep.tile([128, 2 * BC], BF16)  # rounded copy fed to matmuls
            nc.vector.tensor_copy(zTr[:], zT[:])
            zsT = statep.tile([128, L * 2 * BC], BF16)  # all L z-snapshots
            nc.vector.tensor_copy(zsT[:, 0 : 2 * BC], zT[:])

            # ---- HAM warmup: dummy matmuls keep PE busy ~4-6us so the clock
            # gate opens to 2.4 GHz before the scan starts.
            gP = psG.tile([128, NG * GF], F32, tag="gP")
            for i in range(warmup):
                nc.tensor.matmul(
                    gP[0:BC, 0:GF],
                    onest[:],
                    w2[:, 0:GF],
                    start=True,
                    stop=True,
                    skip_group_check=True,
                )

            def n_jj(g):
                return 4 if g < 4 else 1

            for k in range(n_substeps):
                # ---- 1. bias_k at the FIFO head: executes during substep
                # k-1's tanh/transpose tail (gP regions free as soon as the
                # matching tanh chunk of k-1 has read them).
                gP = psG.tile([128, NG * GF], F32, tag="gP")
                for g in GORDER:
                    for jj in range(n_jj(g)):
                        c = 4 * g + jj
                        nc.tensor.matmul(
                            gP[32 * jj : 32 * jj + 32, g * GF : (g + 1) * GF],
                            onest[:],
                            b2[:, c * 256 : (c + 1) * 256],
                            start=True,
                            stop=False,
                            tile_position=(0, 32 * jj),
                            skip_group_check=True,
                        )

                # ---- 2. dk: block-diag dX built on GPSIMD (engine is idle)
                dk = work.tile([128, NG * BC], BF16, tag="dk")
                for g in range(NG):
                    nc.gpsimd.tensor_scalar(
                        dk[:, g * BC : (g + 1) * BC],
                        maskt[:, g * BC : (g + 1) * BC],
                        dcolt[:, k * NG + g : k * NG + g + 1],
                        None,
                        AluOp.mult,
                    )

                # ---- 3. GEMM1: hdnT = (z @ W1)^T via col-tiled strips;
                # kh (= zTr half) outer so kh0 strips start as soon as the
                # first half of the z-update lands.
                hdnP = psH.tile([128, 2 * BC], F32, tag="hdnP")
                for kh in range(2):
                    for mh in range(2):
                        for jj in range(4):
                            nc.tensor.matmul(
                                hdnP[32 * jj : 32 * jj + 32, mh * BC : (mh + 1) * BC],
                                w1[:, (kh * 2 + mh) * 128 + 32 * jj : (kh * 2 + mh) * 128 + 32 * jj + 32],
                                zTr[:, kh * BC : (kh + 1) * BC],
                                start=(kh == 0),
                                stop=(kh == 1),
                                tile_position=(0, 32 * jj),
                            )
                # relu(x + b1), per h-half (bias is per-partition within a half)
                hdn = work.tile([128, 2 * BC], BF16, tag="hdn")
                for mh in range(2):
                    nc.vector.tensor_scalar(
                        hdn[:, mh * BC : (mh + 1) * BC],
                        hdnP[:, mh * BC : (mh + 1) * BC],
                        b1[:, mh : mh + 1],
                        0.0,
                        AluOp.add,
                        AluOp.max,
                    )

                # ---- 4. mains + eager per-chunk tanh ----
                gS = work.tile([128, NG * GF], BF16, tag="gS")

                def emit_g(g):
                    for kh in range(2):
                        for jj in range(n_jj(g)):
                            c = 4 * g + jj
                            nc.tensor.matmul(
                                gP[32 * jj : 32 * jj + 32, g * GF : (g + 1) * GF],
                                hdn[:, kh * BC : (kh + 1) * BC],
                                w2[:, kh * C * 256 + c * 256 : kh * C * 256 + c * 256 + 256],
                                start=False,
                                stop=(kh == 1),
                                tile_position=(0, 32 * jj),
                                skip_group_check=True,
                            )

                done = set()
                for g in GORDER:
                    emit_g(g)
                    done.add(g)
                    for lo, hi, gs in TANH_CHUNKS:
                        if all(x in done for x in gs) and (lo, hi) not in getattr(
                            emit_g, "_flushed", set()
                        ):
                            fl = getattr(emit_g, "_flushed", set())
                            fl.add((lo, hi))
                            emit_g._flushed = fl
                            nc.scalar.activation(gS[:, lo:hi], gP[:, lo:hi], Act.Tanh)
                emit_g._flushed = set()

                # ---- 5. contracts, h-half split, AFTER all mains in the PE
                # FIFO (their tanh inputs are ready by the time they issue).
                zdP = psZ.tile([BC, 256], F32, tag="zdP")
                for hh in range(2):
                    for i, g in enumerate(GORDER):
                        lhs = dk[:, g * BC : (g + 1) * BC] if g < 4 else dk[0:BC, g * BC : (g + 1) * BC]
                        rhs = (
                            gS[:, g * GF + hh * 128 : g * GF + hh * 128 + 128]
                            if g < 4
                            else gS[0:BC, g * GF + hh * 128 : g * GF + hh * 128 + 128]
                        )
                        nc.tensor.matmul(
                            zdP[:, hh * 128 : hh * 128 + 128],
                            lhs,
                            rhs,
                            start=(i == 0),
                            stop=(i == NG - 1),
                            skip_group_check=True,
                        )
                    # PSUM->SBUF copy of this half pipelines under the other
                    # half's contracts
                    if hh == 0:
                        zd = work.tile([BC, 256], F32, tag="zd")
                    nc.vector.tensor_copy(
                        zd[:, hh * 128 : hh * 128 + 128],
                        zdP[:, hh * 128 : hh * 128 + 128],
                    )

                # ---- 6. transpose halves + state update ----
                zdT = psZ.tile([128, 2 * BC], F32, tag="zdT")
                for hh in range(2):
                    nc.tensor.transpose(
                        zdT[:, hh * BC : (hh + 1) * BC],
                        zd[:, hh * 128 : (hh + 1) * 128],
                        identt[:],
                    )
                    # rounded bf16 state per half (critical path to GEMM1 kh)
                    nc.vector.tensor_add(
                        zTr[:, hh * BC : (hh + 1) * BC],
                        zT[:, hh * BC : (hh + 1) * BC],
                        zdT[:, hh * BC : (hh + 1) * BC],
                    )
                # fp32 master off the critical path
                nc.vector.tensor_add(zT[:], zT[:], zdT[:])

                if dump and k == 0:
                    dbg_hdn_s = work.tile([128, 2 * BC], F32, tag="dbg1")
                    nc.vector.tensor_copy(dbg_hdn_s[:], hdn[:])
                    nc.sync.dma_start(dbg_hdn[:], dbg_hdn_s[:])
                    dbg_g_s = work.tile([128, NG * GF], F32, tag="dbg2")
                    nc.vector.tensor_copy(dbg_g_s[:], gS[:])
                    nc.sync.dma_start(dbg_g[:], dbg_g_s[:])
                    nc.sync.dma_start(dbg_zd[:], zd[:])
                    nc.sync.dma_start(dbg_zt[:], zT[:])

                if k % 2 == 1:
                    step = k // 2
                    nc.vector.tensor_copy(
                        zsT[:, (step + 1) * 2 * BC : (step + 2) * 2 * BC], zTr[:]
                    )

        # ---- decode: out[l, b, y] = zs[l, b, :] @ dec_W + dec_b ----
        with (
            tc.tile_pool(name="psD", bufs=1, space="PSUM") as psD,
            tc.tile_pool(name="od", bufs=1) as odp,
        ):
            zs3 = zsT[:].rearrange("p (e x) -> p e x", x=2 * BC)
            outP = psD.tile([Y, 4096], F32)
            n_sc = L // 8  # 16 step-chunks of 8 entries
            for sc in range(n_sc):
                for hh in range(2):
                    nc.tensor.matmul(
                        outP[:, sc * 256 : (sc + 1) * 256],
                        decwt[:, hh * Y : (hh + 1) * Y],
                        zs3[:, sc * 8 : (sc + 1) * 8, hh * BC : (hh + 1) * BC],
                        start=(hh == 0),
                        stop=(hh == 1),
                        skip_group_check=True,
                    )
            outS = odp.tile([Y, 4096], F32)
            nc.vector.tensor_scalar(
                outS[:], outP[:], decbt[0:Y, 0:1], None, AluOp.add
            )
            outv = out[:].rearrange("(sc s) b y -> sc y s b", s=8)
            for sc in range(n_sc):
                src_ap = outS[:, sc * 256 : (sc + 1) * 256]
                nc.sync.dma_start(outv[sc], src_ap)

    nc.compile()
    return nc


def host_prep(ts, us, enc_b, f_W1, f_b1, f_W2, f_b2, dec_W, dec_b, n_substeps=NSUBSTEPS):
    """Host-side packing of weights + spline-derivative scalars."""
    ts = np.asarray(ts, np.float64)
    us = np.asarray(us, np.float64)
    t = ts[:, 0, 0]
    dt = t[1:] - t[:-1]                                  # (L-1,)
    x = np.concatenate([ts, us], axis=-1).transpose(1, 0, 2)  # (B, L, C)
    h = dt[None, :, None]
    slope = (x[:, 1:] - x[:, :-1]) / h
    m = np.concatenate([slope[:, :1], slope], axis=1)
    mi, mn = m[:, :-1], m[:, 1:]
    xi, xn = x[:, :-1], x[:, 1:]
    c2 = 3.0 * (xn - xi) / h**2 - (2.0 * mi + mn) / h
    c3 = 2.0 * (xi - xn) / h**3 + (mi + mn) / h**2
    dX0 = mi                                             # u = 0
    dX1 = mi + c2 * h + 0.75 * c3 * h * h                # u = h/2
    scale = h / N_SUB                                    # (1, L-1, 1)
    dxs = np.stack([dX0 * scale, dX1 * scale], axis=2)   # (B, L-1, 2, C)
    dxs = dxs.transpose(1, 2, 0, 3).reshape(NSUBSTEPS, B, C).astype(np.float32)

    f_W1 = np.asarray(f_W1, np.float32)
    f_W2 = np.asarray(f_W2, np.float32)
    f_b1 = np.asarray(f_b1, np.float32)
    f_b2 = np.asarray(f_b2, np.float32)
    enc_b = np.asarray(enc_b, np.float32)
    dec_W = np.asarray(dec_W, np.float32)
    dec_b = np.asarray(dec_b, np.float32)

    # W1 packed: w1s[p, (kh*2+mh)*128 + m] = W1[kh*128+p, mh*128+m]
    w1s = np.zeros((128, 512), np.float32)
    for kh in range(2):
        for mh in range(2):
            w1s[:, (kh * 2 + mh) * 128 : (kh * 2 + mh + 1) * 128] = f_W1[
                kh * 128 : (kh + 1) * 128, mh * 128 : (mh + 1) * 128
            ]

    # W2 c-major (no padding): w2s[p, kh*C*256 + c*256 + h2] = W2[kh*128+p, h2*C + c]
    w2r = f_W2.reshape(H, H, C)                          # [h_in, h_out, c]
    w2cm = w2r.transpose(0, 2, 1).reshape(H, C * H)      # [h_in, c, h_out]
    w2s = np.concatenate([w2cm[:128], w2cm[128:]], axis=1)  # (128, 2*C*256)

    b2r = f_b2.reshape(H, C)
    b2cm = b2r.T.reshape(1, C * H)                       # [c, h_out]
    b2s = np.broadcast_to(b2cm, (128, C * H)).copy()

    b1t = np.stack([f_b1[:128], f_b1[128:]], axis=1).astype(np.float32)  # (128, 2)

    z0 = enc_b                                            # zeros @ enc_W + enc_b
    zt0 = np.zeros((128, 2 * BC), np.float32)
    for hh in range(2):
        zt0[:, hh * BC : (hh + 1) * BC] = z0[hh * 128 : (hh + 1) * 128][:, None]

    # mask[32*jj + bb, g*BC + bb'] = (bb == bb') for groups with c = 4g+jj < C
    maskd = np.zeros((128, NG * BC), np.float32)
    bb = np.arange(BC)
    for g in range(NG):
        for jj in range(4 if g < 4 else 1):
            maskd[32 * jj + bb, g * BC + bb] = 1.0

    # dcol[32*jj + bb, k*NG + g] = dxs[k, core*BC + bb, 4g+jj]
    dcol_cores = []
    for core in range(NCORES):
        d = np.zeros((128, NSUBSTEPS * NG), np.float32)
        for g in range(NG):
            for jj in range(4 if g < 4 else 1):
                c = 4 * g + jj
                d[32 * jj + bb[:, None], np.arange(n_substeps)[None, :] * NG + g] = dxs[
                    :n_substeps, core * BC + bb, c
                ].T
        dcol_cores.append(d)

    decw = np.concatenate([dec_W[:128], dec_W[128:]], axis=1).astype(np.float32)  # (128, 2Y)
    decb = np.zeros((128, 1), np.float32)
    for jj in range(4):
        decb[32 * jj : 32 * jj + Y, 0] = dec_b

    common = {
        "w1s": w1s.astype(ml_dtypes.bfloat16),
        "w2s": w2s.astype(ml_dtypes.bfloat16),
        "b2s": b2s.astype(ml_dtypes.bfloat16),
        "b1t": b1t,
        "zt0": zt0,
        "maskd": maskd.astype(ml_dtypes.bfloat16),
        "ones": np.eye(128, BC, dtype=ml_dtypes.bfloat16),
        "ident": np.eye(BC, dtype=np.float32),
        "decw": decw.astype(ml_dtypes.bfloat16),
        "decb": decb,
    }
    in_maps = []
    for core in range(NCORES):
        m_ = dict(common)
        m_["dcol"] = dcol_cores[core]
        in_maps.append(m_)
    return in_maps


_CACHE = {}


def _get_nc(n_substeps=NSUBSTEPS, dump=False):
    key = (n_substeps, dump)
    if key not in _CACHE:
        _CACHE[key] = build_bass(n_substeps, dump=dump)
    return _CACHE[key]


def run(inputs, n_substeps=NSUBSTEPS, trace=False, dump=False, **kw):
    in_maps = host_prep(
        inputs["ts"], inputs["us"], inputs["enc_b"], inputs["f_W1"],
        inputs["f_b1"], inputs["f_W2"], inputs["f_b2"], inputs["dec_W"],
        inputs["dec_b"], n_substeps=n_substeps,
    )
    nc = _get_nc(n_substeps, dump)
    res = run_bass_kernel_spmd(nc, in_maps, core_ids=list(range(NCORES)), trace=trace, **kw)
    outs = [np.asarray(res.results[i]["out"]) for i in range(NCORES)]
    full = np.concatenate(outs, axis=1)  # (L, B, Y)
    return full, res


def kernel(**inputs) -> np.ndarray:
    full, _ = run(inputs)
    return full.astype(np.float32)


# revision 6
# speedup vs baseline: 1.7256x; 1.7256x over previous
"""Trainium2 Bass kernel for a Neural CDE (GunnarODE).

Full-input contract: kernel(**inputs) takes the complete (unsharded) inputs
and returns the complete (L, B, Y) output. Internally the batch dim (B=256)
is sharded across 8 NeuronCores (32 per core); the small MLP weights are
replicated. The sequential 127-step x 2-substep Euler scan runs on-device.

v4 design notes:
- Dependency tracking is tile-granular, so every producer/consumer pair that
  must overlap gets its own tile: gP/gS are split per tanh-chunk (A=[c16],
  B=[g0,g1], C=[g2,g3]), the contraction PSUM, its SBUF copy, the transposed
  delta, the rounded state and the hidden activations are all split by half.
- Cross-iteration software pipelining: substep k's bias matmuls are emitted
  before substep k-1's tail (contract g2/g3, transposes, state round), so
  the PE queue never drains while the scalar engine finishes tanh.
- Zero per-substep DMA: spline-derivative scalars stay resident; the
  block-diag dX operand is rebuilt each substep with one broadcast DVE op.
- fp32 master state; bf16 tanh outputs, dX, snapshots and decoder.
"""

import sys

for _p in ("/opt/trn_rl_repo", "/root/.axon_site/_ro/trn_rl_repo"):
    if _p not in sys.path:
        sys.path.append(_p)

import numpy as np
import ml_dtypes

import concourse.bass as bass
import concourse.bacc as bacc
import concourse.mybir as mybir
from concourse.tile import TileContext
from concourse.bass_utils import run_bass_kernel_spmd

# Problem dims (hardcoded per contract)
L, B, H, C, Y = 128, 256, 256, 17, 16
N_SUB = 2
NCORES = 8
BC = B // NCORES           # batch per core = 32
NSTEPS = L - 1             # 127 scan steps
NSUBSTEPS = NSTEPS * N_SUB # 254
NG = 5                     # c-groups: [4,4,4,4,1] (g4 = c16 singleton)
GF = 256                   # free width of one c-group block in G
F32 = mybir.dt.float32
BF16 = mybir.dt.bfloat16

AluOp = mybir.AluOpType
Act = mybir.ActivationFunctionType

# chunk -> (groups); group -> (chunk, offset within chunk tile)
CHUNKS = {"A": [4], "B": [0, 1], "C": [2, 3]}
GPLACE = {4: ("A", 0), 0: ("B", 0), 1: ("B", GF), 2: ("C", 0), 3: ("C", GF)}


def build_bass(n_substeps=NSUBSTEPS, warmup=40, dump=False):
    """Build the per-core Bass program (same program for all cores)."""
    nc = bacc.Bacc("TRN2", target_bir_lowering=False, debug=False)

    w1s = nc.dram_tensor("w1s", [128, 512], BF16, kind="ExternalInput")
    w2s = nc.dram_tensor("w2s", [128, 2 * C * 256], BF16, kind="ExternalInput")
    b2s = nc.dram_tensor("b2s", [128, C * 256], BF16, kind="ExternalInput")
    b1t = nc.dram_tensor("b1t", [128, 2], F32, kind="ExternalInput")
    zt0 = nc.dram_tensor("zt0", [128, 2 * BC], F32, kind="ExternalInput")
    dcol = nc.dram_tensor("dcol", [128, NSUBSTEPS * NG], F32, kind="ExternalInput")
    maskd = nc.dram_tensor("maskd", [128, NG * BC], BF16, kind="ExternalInput")
    ones = nc.dram_tensor("ones", [128, BC], BF16, kind="ExternalInput")
    ident = nc.dram_tensor("ident", [BC, BC], F32, kind="ExternalInput")
    decw = nc.dram_tensor("decw", [128, 2 * Y], BF16, kind="ExternalInput")
    decb = nc.dram_tensor("decb", [128, 1], F32, kind="ExternalInput")
    out = nc.dram_tensor("out", [L, BC, Y], F32, kind="ExternalOutput")

    with TileContext(nc) as tc:
        with (
            tc.tile_pool(name="const", bufs=1) as constp,
            tc.tile_pool(name="work", bufs=2) as work,
            tc.tile_pool(name="state", bufs=1) as statep,
            tc.tile_pool(name="psG", bufs=1, space="PSUM") as psG,
            tc.tile_pool(name="psH", bufs=1, space="PSUM") as psH,
            tc.tile_pool(name="psZ", bufs=1, space="PSUM") as psZ,
        ):
            w1 = constp.tile([128, 512], BF16)
            nc.sync.dma_start(w1[:], w1s[:])
            w2 = constp.tile([128, 2 * C * 256], BF16)
            nc.sync.dma_start(w2[:], w2s[:])
            b2 = constp.tile([128, C * 256], BF16)
            nc.sync.dma_start(b2[:], b2s[:])
            b1 = constp.tile([128, 2], F32)
            nc.sync.dma_start(b1[:], b1t[:])
            onest = constp.tile([128, BC], BF16)
            nc.sync.dma_start(onest[:], ones[:])
            identt = constp.tile([BC, BC], F32)
            nc.sync.dma_start(identt[:], ident[:])
            maskt = constp.tile([128, NG * BC], BF16)
            nc.sync.dma_start(maskt[:], maskd[:])
            dcolt = constp.tile([128, NSUBSTEPS * NG], F32)
            nc.sync.dma_start(dcolt[:], dcol[:])
            decwt = constp.tile([128, 2 * Y], BF16)
            nc.sync.dma_start(decwt[:], decw[:])
            decbt = constp.tile([128, 1], F32)
            nc.sync.dma_start(decbt[:], decb[:])

            # split state: per h-half tiles so each half's round unlocks its
            # GEMM1 strips independently
            zTh = [statep.tile([128, BC], F32, tag=f"zT{h}", name=f"zT{h}") for h in range(2)]
            zTrh = [statep.tile([128, BC], BF16, tag=f"zTr{h}", name=f"zTr{h}") for h in range(2)]
            for h in range(2):
                nc.sync.dma_start(zTh[h][:], zt0[:, h * BC : (h + 1) * BC])
                nc.vector.tensor_copy(zTrh[h][:], zTh[h][:])
            zsT = statep.tile([128, L * 2 * BC], BF16)  # all L z-snapshots
            for h in range(2):
                nc.vector.tensor_copy(zsT[:, h * BC : (h + 1) * BC], zTh[h][:])

            # ---- HAM warmup: dummy matmuls keep PE busy ~4-6us so the clock
            # gate opens to 2.4 GHz before the scan starts.
            wuP = psG.tile([128, GF], F32, tag="gPA")
            for i in range(warmup):
                nc.tensor.matmul(
                    wuP[0:BC, 0:GF],
                    onest[:],
                    w2[:, 0:GF],
                    start=True,
                    stop=True,
                    skip_group_check=True,
                )

            def n_jj(g):
                return 4 if g < 4 else 1

            def emit_bias(g, gP, off):
                for jj in range(n_jj(g)):
                    c = 4 * g + jj
                    nc.tensor.matmul(
                        gP[32 * jj : 32 * jj + 32, off : off + GF],
                        onest[:],
                        b2[:, c * 256 : (c + 1) * 256],
                        start=True,
                        stop=False,
                        tile_position=(0, 32 * jj),
                        skip_group_check=True,
                    )

            def emit_mains(g, gP, off, hdnh):
                for kh in range(2):
                    for jj in range(n_jj(g)):
                        c = 4 * g + jj
                        nc.tensor.matmul(
                            gP[32 * jj : 32 * jj + 32, off : off + GF],
                            hdnh[kh][:],
                            w2[:, kh * C * 256 + c * 256 : kh * C * 256 + c * 256 + 256],
                            start=False,
                            stop=(kh == 1),
                            tile_position=(0, 32 * jj),
                            skip_group_check=True,
                        )

            def emit_contract(g, hh, gchunk, zdPh, start, stop, dkt):
                _, off = GPLACE[g]
                if g < 4:
                    lhs = dkt[:, g * BC : (g + 1) * BC]
                    rhs = gchunk[:, off + hh * 128 : off + hh * 128 + 128]
                else:
                    lhs = dkt[0:BC, g * BC : (g + 1) * BC]
                    rhs = gchunk[0:BC, off + hh * 128 : off + hh * 128 + 128]
                nc.tensor.matmul(
                    zdPh[:], lhs, rhs, start=start, stop=stop, skip_group_check=True
                )

            def emit_prev_tail_contracts(pv):
                # finish substep k-1: contracts for chunk C groups, both
                # halves, with the PSUM->SBUF copy right behind each half
                for hh in range(2):
                    emit_contract(2, hh, pv["gSC"], pv["zdP"][hh], False, False, pv["dk"])
                    emit_contract(3, hh, pv["gSC"], pv["zdP"][hh], False, True, pv["dk"])
                    nc.vector.tensor_copy(pv["zd"][hh][:], pv["zdP"][hh][:])

            def emit_prev_transp_round(pv):
                zdT = pv["zdT"]
                for hh in range(2):
                    nc.tensor.transpose(
                        zdT[:, hh * BC : (hh + 1) * BC], pv["zd"][hh][:], identt[:]
                    )
                for hh in range(2):
                    # rounded bf16 state per half (critical path to GEMM1)
                    nc.vector.tensor_add(
                        zTrh[hh][:], zTh[hh][:], zdT[:, hh * BC : (hh + 1) * BC]
                    )

            def emit_prev_master(pv):
                for hh in range(2):
                    nc.vector.tensor_add(
                        zTh[hh][:], zTh[hh][:], pv["zdT"][:, hh * BC : (hh + 1) * BC]
                    )
                if pv["k"] % 2 == 1:
                    step = pv["k"] // 2
                    for hh in range(2):
                        nc.gpsimd.tensor_copy(
                            zsT[:, (step + 1) * 2 * BC + hh * BC : (step + 1) * 2 * BC + (hh + 1) * BC],
                            zTrh[hh][:],
                        )

            prev = None
            for k in range(n_substeps):
                gPA = psG.tile([128, GF], F32, tag="gPA")
                gPB = psG.tile([128, 2 * GF], F32, tag="gPB")
                gPC = psG.tile([128, 2 * GF], F32, tag="gPC")
                gtile = {"A": gPA, "B": gPB, "C": gPC}

                # 1. bias for chunks A, B (covers k-1's tanh-C wait)
                emit_bias(4, gPA, 0)
                emit_bias(0, gPB, 0)
                emit_bias(1, gPB, GF)

                # 2. k-1 tail contracts + copies
                if prev is not None:
                    emit_prev_tail_contracts(prev)

                # 3. bias for chunk C
                emit_bias(2, gPC, 0)
                emit_bias(3, gPC, GF)

                # 4/5. k-1 transposes + state round
                if prev is not None:
                    emit_prev_transp_round(prev)

                # 6. GEMM1 (kh outer: kh0 strips only need zTr half 0)
                hdnP = psH.tile([128, 2 * BC], F32, tag="hdnP", name="hdnP")
                for kh in range(2):
                    for mh in range(2):
                        for jj in range(4):
                            nc.tensor.matmul(
                                hdnP[32 * jj : 32 * jj + 32, mh * BC : (mh + 1) * BC],
                                w1[:, (kh * 2 + mh) * 128 + 32 * jj : (kh * 2 + mh) * 128 + 32 * jj + 32],
                                zTrh[kh][:],
                                start=(kh == 0),
                                stop=(kh == 1),
                                tile_position=(0, 32 * jj),
                            )

                # 7. relu per half
                hdnh = [work.tile([128, BC], BF16, tag=f"hdn{m}", name=f"hdn{m}") for m in range(2)]
                for mh in range(2):
                    nc.vector.tensor_scalar(
                        hdnh[mh][:],
                        hdnP[:, mh * BC : (mh + 1) * BC],
                        b1[:, mh : mh + 1],
                        0.0,
                        AluOp.add,
                        AluOp.max,
                    )

                # 8/9. k-1 master update + snapshot (off critical path)
                if prev is not None:
                    emit_prev_master(prev)

                # 10. dk for this substep: one broadcast DVE op
                dk = work.tile([128, NG * BC], BF16, tag="dk")
                dk3 = dk[:].rearrange("p (g b) -> p g b", b=BC)
                mask3 = maskt[:].rearrange("p (g b) -> p g b", b=BC)
                dc3 = (
                    dcolt[:, k * NG : (k + 1) * NG]
                    .unsqueeze(2)
                    .broadcast_to([128, NG, BC])
                )
                nc.vector.tensor_tensor(dk3, mask3, dc3, AluOp.mult)

                # 11. mains + eager per-chunk tanh
                gSA = work.tile([128, GF], BF16, tag="gSA")
                gSB = work.tile([128, 2 * GF], BF16, tag="gSB")
                gSC = work.tile([128, 2 * GF], BF16, tag="gSC")
                emit_mains(4, gPA, 0, hdnh)
                nc.scalar.activation(gSA[:], gPA[:], Act.Tanh)
                emit_mains(0, gPB, 0, hdnh)
                emit_mains(1, gPB, GF, hdnh)
                nc.scalar.activation(gSB[:], gPB[:], Act.Tanh)
                emit_mains(2, gPC, 0, hdnh)
                emit_mains(3, gPC, GF, hdnh)
                nc.scalar.activation(gSC[:], gPC[:], Act.Tanh)

                # 12. contracts for chunks A, B (tanh done during mains)
                zdPh = [psZ.tile([BC, 128], F32, tag=f"zdP{h}", name=f"zdP{h}") for h in range(2)]
                for hh in range(2):
                    emit_contract(4, hh, gSA, zdPh[hh], True, False, dk)
                    emit_contract(0, hh, gSB, zdPh[hh], False, False, dk)
                    emit_contract(1, hh, gSB, zdPh[hh], False, False, dk)

                prev = {
                    "k": k,
                    "dk": dk,
                    "gSC": gSC,
                    "zdP": zdPh,
                    "zd": [work.tile([BC, 128], F32, tag=f"zd{h}", name=f"zd{h}") for h in range(2)],
                    "zdT": psZ.tile([128, 2 * BC], F32, tag="zdT", name="zdT"),
                }

            # epilogue: finish the last substep
            emit_prev_tail_contracts(prev)
            emit_prev_transp_round(prev)
            emit_prev_master(prev)

        # ---- decode: out[l, b, y] = zs[l, b, :] @ dec_W + dec_b ----
        with (
            tc.tile_pool(name="psD", bufs=1, space="PSUM") as psD,
            tc.tile_pool(name="od", bufs=1) as odp,
        ):
            zs3 = zsT[:].rearrange("p (e x) -> p e x", x=2 * BC)
            outP = psD.tile([Y, 4096], F32)
            n_sc = L // 8  # 16 step-chunks of 8 entries
            for sc in range(n_sc):
                for hh in range(2):
                    nc.tensor.matmul(
                        outP[:, sc * 256 : (sc + 1) * 256],
                        decwt[:, hh * Y : (hh + 1) * Y],
                        zs3[:, sc * 8 : (sc + 1) * 8, hh * BC : (hh + 1) * BC],
                        start=(hh == 0),
                        stop=(hh == 1),
                        skip_group_check=True,
                    )
            outS = odp.tile([Y, 4096], F32)
            nc.vector.tensor_scalar(
                outS[:], outP[:], decbt[0:Y, 0:1], None, AluOp.add
            )
            outv = out[:].rearrange("(sc s) b y -> sc y s b", s=8)
            for sc in range(n_sc):
                src_ap = outS[:, sc * 256 : (sc + 1) * 256]
                nc.sync.dma_start(outv[sc], src_ap)

    nc.compile()
    return nc


def host_prep(ts, us, enc_b, f_W1, f_b1, f_W2, f_b2, dec_W, dec_b, n_substeps=NSUBSTEPS):
    """Host-side packing of weights + spline-derivative scalars."""
    ts = np.asarray(ts, np.float64)
    us = np.asarray(us, np.float64)
    t = ts[:, 0, 0]
    dt = t[1:] - t[:-1]                                  # (L-1,)
    x = np.concatenate([ts, us], axis=-1).transpose(1, 0, 2)  # (B, L, C)
    h = dt[None, :, None]
    slope = (x[:, 1:] - x[:, :-1]) / h
    m = np.concatenate([slope[:, :1], slope], axis=1)
    mi, mn = m[:, :-1], m[:, 1:]
    xi, xn = x[:, :-1], x[:, 1:]
    c2 = 3.0 * (xn - xi) / h**2 - (2.0 * mi + mn) / h
    c3 = 2.0 * (xi - xn) / h**3 + (mi + mn) / h**2
    dX0 = mi                                             # u = 0
    dX1 = mi + c2 * h + 0.75 * c3 * h * h                # u = h/2
    scale = h / N_SUB                                    # (1, L-1, 1)
    dxs = np.stack([dX0 * scale, dX1 * scale], axis=2)   # (B, L-1, 2, C)
    dxs = dxs.transpose(1, 2, 0, 3).reshape(NSUBSTEPS, B, C).astype(np.float32)

    f_W1 = np.asarray(f_W1, np.float32)
    f_W2 = np.asarray(f_W2, np.float32)
    f_b1 = np.asarray(f_b1, np.float32)
    f_b2 = np.asarray(f_b2, np.float32)
    enc_b = np.asarray(enc_b, np.float32)
    dec_W = np.asarray(dec_W, np.float32)
    dec_b = np.asarray(dec_b, np.float32)

    # W1 packed: w1s[p, (kh*2+mh)*128 + m] = W1[kh*128+p, mh*128+m]
    w1s = np.zeros((128, 512), np.float32)
    for kh in range(2):
        for mh in range(2):
            w1s[:, (kh * 2 + mh) * 128 : (kh * 2 + mh + 1) * 128] = f_W1[
                kh * 128 : (kh + 1) * 128, mh * 128 : (mh + 1) * 128
            ]

    # W2 c-major (no padding): w2s[p, kh*C*256 + c*256 + h2] = W2[kh*128+p, h2*C + c]
    w2r = f_W2.reshape(H, H, C)                          # [h_in, h_out, c]
    w2cm = w2r.transpose(0, 2, 1).reshape(H, C * H)      # [h_in, c, h_out]
    w2s = np.concatenate([w2cm[:128], w2cm[128:]], axis=1)  # (128, 2*C*256)

    b2r = f_b2.reshape(H, C)
    b2cm = b2r.T.reshape(1, C * H)                       # [c, h_out]
    b2s = np.broadcast_to(b2cm, (128, C * H)).copy()

    b1t = np.stack([f_b1[:128], f_b1[128:]], axis=1).astype(np.float32)  # (128, 2)

    z0 = enc_b                                            # zeros @ enc_W + enc_b
    zt0 = np.zeros((128, 2 * BC), np.float32)
    for hh in range(2):
        zt0[:, hh * BC : (hh + 1) * BC] = z0[hh * 128 : (hh + 1) * 128][:, None]

    # mask[32*jj + bb, g*BC + bb'] = (bb == bb') for groups with c = 4g+jj < C
    maskd = np.zeros((128, NG * BC), np.float32)
    bb = np.arange(BC)
    for g in range(NG):
        for jj in range(4 if g < 4 else 1):
            maskd[32 * jj + bb, g * BC + bb] = 1.0

    # dcol[32*jj + bb, k*NG + g] = dxs[k, core*BC + bb, 4g+jj]
    dcol_cores = []
    for core in range(NCORES):
        d = np.zeros((128, NSUBSTEPS * NG), np.float32)
        for g in range(NG):
            for jj in range(4 if g < 4 else 1):
                c = 4 * g + jj
                d[32 * jj + bb[:, None], np.arange(n_substeps)[None, :] * NG + g] = dxs[
                    :n_substeps, core * BC + bb, c
                ].T
        dcol_cores.append(d)

    decw = np.concatenate([dec_W[:128], dec_W[128:]], axis=1).astype(np.float32)  # (128, 2Y)
    decb = np.zeros((128, 1), np.float32)
    for jj in range(4):
        decb[32 * jj : 32 * jj + Y, 0] = dec_b

    common = {
        "w1s": w1s.astype(ml_dtypes.bfloat16),
        "w2s": w2s.astype(ml_dtypes.bfloat16),
        "b2s": b2s.astype(ml_dtypes.bfloat16),
        "b1t": b1t,
        "zt0": zt0,
        "maskd": maskd.astype(ml_dtypes.bfloat16),
        "ones": np.eye(128, BC, dtype=ml_dtypes.bfloat16),
        "ident": np.eye(BC, dtype=np.float32),
        "decw": decw.astype(ml_dtypes.bfloat16),
        "decb": decb,
    }
    in_maps = []
    for core in range(NCORES):
        m_ = dict(common)
        m_["dcol"] = dcol_cores[core]
        in_maps.append(m_)
    return in_maps


_CACHE = {}


def _get_nc(n_substeps=NSUBSTEPS):
    key = n_substeps
    if key not in _CACHE:
        _CACHE[key] = build_bass(n_substeps)
    return _CACHE[key]


def run(inputs, n_substeps=NSUBSTEPS, trace=False, **kw):
    in_maps = host_prep(
        inputs["ts"], inputs["us"], inputs["enc_b"], inputs["f_W1"],
        inputs["f_b1"], inputs["f_W2"], inputs["f_b2"], inputs["dec_W"],
        inputs["dec_b"], n_substeps=n_substeps,
    )
    nc = _get_nc(n_substeps)
    res = run_bass_kernel_spmd(nc, in_maps, core_ids=list(range(NCORES)), trace=trace, **kw)
    outs = [np.asarray(res.results[i]["out"]) for i in range(NCORES)]
    full = np.concatenate(outs, axis=1)  # (L, B, Y)
    return full, res


def kernel(**inputs) -> np.ndarray:
    full, _ = run(inputs)
    return full.astype(np.float32)
